# revision 1
# baseline (speedup 1.0000x reference)
"""Conv2d(128->256, 3x3, pad=1) + sync-BatchNorm(train) + ReLU on 8 TRN2 cores.

Strategy (data-parallel, hardcoded for x:[32,128,56,56] w:[256,128,3,3]):
  - Shard batch 32 -> 4 images/core across 8 cores.
  - Host pre-pads x to 58x58 and pre-transposes weights to [Cin, o_tile, o, tap]
    so every device DMA is contiguous.
  - Conv = implicit GEMM: Cin=128 is the partition/contraction dim; each 3x3 tap
    is one float32r matmul ([128,128] weights x [128,448] shifted-image view)
    accumulated in PSUM. Output rows are processed in 7 groups of 8 rows
    (8*56=448 <= 512 fp32 PSUM bank).
  - BN train-mode: bias cancels exactly ((y+b) - mean(y+b) == y - mean(y)).
    Per-channel partial sums/sumsq are folded into PSUM evacuation:
    ACT Copy w/ accum_out (sum) + DVE tensor_tensor_reduce square (sumsq).
    A [128,4] AllReduce across the 8 cores provides sync-BN semantics.
  - Final: out = Relu(y*scale + shift), one fused ACT op per (o_tile, image),
    DMA'd straight to DRAM.
"""

import numpy as np

import concourse.bass as bass
import concourse.mybir as mybir
import concourse.tile as tile
from concourse import bacc

F32 = mybir.dt.float32
F32R = mybir.dt.float32r

N_CORES = 8
IMGS = 4            # images per core
CIN = 128
COUT = 256
H = W = 56
HP = WP = 58        # padded
NG = 7              # row-groups per image (8 rows each)
RG = 8              # rows per group
GROUP = RG * W      # 448
BANK = 512          # fp32 elems per PSUM bank
EPS = 1e-5
COUNT = float(32 * H * W)   # global BN element count per channel

AF = mybir.ActivationFunctionType
ALU = mybir.AluOpType


def build_nc() -> bass.Bass:
    # Bacc (not raw Bass): its compile pipeline legalizes semaphore waits
    # (TRN2 allows at most one wait per instruction; matmul waits move to
    # ldweights / event-semaphore instructions).
    nc = bacc.Bacc()
    xp_d = nc.declare_dram_parameter("xp", [IMGS, CIN, HP, WP], F32R, isOutput=False)
    wt_d = nc.declare_dram_parameter("wt", [CIN, 2, 128, 9], F32R, isOutput=False)
    gb_d = nc.declare_dram_parameter("gb", [128, 4], F32, isOutput=False)
    out_d = nc.declare_dram_parameter("out", [IMGS, COUT, H, W], F32, isOutput=True)

    with tile.TileContext(nc) as tc:
        with (
            tc.tile_pool(name="const", bufs=1) as cpool,
            tc.tile_pool(name="psum", bufs=2, space="PSUM") as ppool,
            tc.tile_pool(name="scrp", bufs=2) as spool,
            tc.tile_pool(name="dram", bufs=1, space="DRAM") as dpool,
        ):
            Wt = cpool.tile([128, 2, 128, 9], F32R)
            GB = cpool.tile([128, 4], F32)
            X = cpool.tile([128, IMGS, HP, WP], F32R)
            Y = cpool.tile([128, 2, IMGS, NG, GROUP], F32)
            Ssum = cpool.tile([128, 2, IMGS * 2], F32)
            Ssq = cpool.tile([128, 2, IMGS * 2], F32)
            SP = cpool.tile([128, 4], F32)
            G = cpool.tile([128, 4], F32)
            mean = cpool.tile([128, 2], F32)
            e2 = cpool.tile([128, 2], F32)
            var = cpool.tile([128, 2], F32)
            std = cpool.tile([128, 2], F32)
            inv = cpool.tile([128, 2], F32)
            sc = cpool.tile([128, 2], F32)
            ms = cpool.tile([128, 2], F32)
            sh = cpool.tile([128, 2], F32)
            epsT = cpool.tile([128, 1], F32)
            bnc_in = dpool.tile([128, 4], F32)
            bnc_out = dpool.tile([128, 4], F32)

            # ---- loads ----
            nc.sync.dma_start(Wt[:, :, :, :], wt_d[:, :, :, :])
            nc.sync.dma_start(GB[:, :], gb_d[:, :])
            for n in range(IMGS):
                nc.sync.dma_start(X[:, n, :, :], xp_d[n, :, :, :])

            # ---- conv + fused partial stats ----
            chunks = [(0, 4), (4, 3)]  # (first group, n groups) -> 4+3 PSUM banks
            for o in range(2):
                for n in range(IMGS):
                    for ci, (g0, ngr) in enumerate(chunks):
                        ps = ppool.tile([128, 4, BANK], F32, tag="ps")
                        for gg in range(ngr):
                            g = g0 + gg
                            for t in range(9):
                                kh, kw = divmod(t, 3)
                                rhs = X[:, n, g * RG + kh : g * RG + kh + RG, kw : kw + W]
                                nc.tensor.matmul(
                                    ps[:, gg, 0:GROUP],
                                    Wt[:, o, :, t],
                                    rhs,
                                    start=(t == 0),
                                    stop=(t == 8),
                                )
                        col = n * 2 + ci
                        # evacuate PSUM -> Y and fold sum(y) into the same ACT op
                        nc.scalar.activation(
                            Y[:, o, n, g0 : g0 + ngr, :],
                            ps[:, 0:ngr, 0:GROUP],
                            AF.Copy,
                            accum_out=Ssum[:, o, col : col + 1],
                        )
                        # sum(y^2) via a second ACT pass (tensor_tensor_reduce
                        # wedges TRN2 here); main out goes to a scratch tile
                        scr = spool.tile([128, 4, GROUP], F32, tag="scr")
                        nc.scalar.activation(
                            scr[:, 0:ngr, :],
                            ps[:, 0:ngr, 0:GROUP],
                            AF.Square,
                            accum_out=Ssq[:, o, col : col + 1],
                        )

            # ---- global stats: pack, all-reduce, finalize ----
            nc.vector.reduce_sum(SP[:, 0:2], Ssum[:, :, :], axis=mybir.AxisListType.X)
            nc.vector.reduce_sum(SP[:, 2:4], Ssq[:, :, :], axis=mybir.AxisListType.X)
            nc.gpsimd.dma_start(bnc_in[:, :], SP[:, :])
            import os as _os

            if _os.environ.get("CONVACT_NO_AR"):
                # debug: skip the collective (per-core stats; output will be
                # slightly wrong vs sync-BN reference)
                nc.gpsimd.dma_start(bnc_out[:, :], bnc_in[:, :])
            else:
                nc.gpsimd.collective_compute(
                    "AllReduce",
                    ALU.add,
                    replica_groups=[list(range(N_CORES))],
                    ins=[bnc_in.opt()],
                    outs=[bnc_out.opt()],
                )
            nc.gpsimd.dma_start(G[:, :], bnc_out[:, :])

            inv_cnt = (N_CORES if _os.environ.get("CONVACT_NO_AR") else 1.0) / COUNT
            nc.vector.tensor_scalar_mul(mean[:, :], G[:, 0:2], inv_cnt)
            nc.vector.tensor_scalar_mul(e2[:, :], G[:, 2:4], inv_cnt)
            nc.vector.memset(epsT[:, :], EPS)
            nc.vector.tensor_mul(var[:, :], mean[:, :], mean[:, :])
            nc.vector.tensor_sub(var[:, :], e2[:, :], var[:, :])
            nc.scalar.activation(std[:, :], var[:, :], AF.Sqrt, bias=epsT[:, 0:1])
            nc.vector.reciprocal(inv[:, :], std[:, :])
            nc.vector.tensor_mul(sc[:, :], GB[:, 0:2], inv[:, :])
            nc.vector.tensor_mul(ms[:, :], mean[:, :], sc[:, :])
            nc.vector.tensor_sub(sh[:, :], GB[:, 2:4], ms[:, :])

            # ---- normalize + relu + store ----
            for o in range(2):
                for n in range(IMGS):
                    ysl = Y[:, o, n, :, :]
                    nc.scalar.activation(
                        ysl,
                        ysl,
                        AF.Relu,
                        bias=sh[:, o : o + 1],
                        scale=sc[:, o : o + 1],
                    )
                    nc.sync.dma_start(
                        out_d[n, o * 128 : (o + 1) * 128, :, :].rearrange(
                            "p h w -> p (h w)"
                        ),
                        ysl.rearrange("p a b -> p (a b)"),
                    )
    return nc


_CACHE: dict = {}


def _get_nc() -> bass.Bass:
    if "nc" not in _CACHE:
        nc = build_nc()
        # Bacc.finalize runs the compile pipeline (wait legalization, register
        # allocation, nop fusion) - required before handing BIR to walrus.
        nc.finalize()
        _CACHE["nc"] = nc
    return _CACHE["nc"]


def _prep_inputs(x, weight, gamma, beta):
    x = np.ascontiguousarray(np.asarray(x, dtype=np.float32))
    w = np.asarray(weight, dtype=np.float32)
    gamma = np.asarray(gamma, dtype=np.float32)
    beta = np.asarray(beta, dtype=np.float32)

    B = x.shape[0]
    per = B // N_CORES
    xp = np.zeros((B, CIN, HP, WP), np.float32)
    xp[:, :, 1 : 1 + H, 1 : 1 + W] = x
    wt = np.ascontiguousarray(w.transpose(1, 0, 2, 3).reshape(CIN, 2, 128, 9))
    gb = np.ascontiguousarray(
        np.stack([gamma[:128], gamma[128:], beta[:128], beta[128:]], axis=1)
    )
    return [
        {"xp": xp[c * per : (c + 1) * per], "wt": wt, "gb": gb}
        for c in range(N_CORES)
    ]


def run(x, weight, bias=None, gamma=None, beta=None, trace=False, **kw):
    """Full-input entry; returns (out, BassKernelResults)."""
    from concourse.bass_utils import run_bass_kernel_spmd

    in_maps = _prep_inputs(x, weight, gamma, beta)
    res = run_bass_kernel_spmd(
        _get_nc(), in_maps, list(range(N_CORES)), trace=trace, **kw
    )
    out = np.concatenate([res.results[c]["out"] for c in range(N_CORES)], axis=0)
    return out, res


def kernel(x, weight, bias=None, gamma=None, beta=None):
    out, _ = run(x, weight, bias=bias, gamma=gamma, beta=beta, trace=False)
    return out



# revision 6
# speedup vs baseline: 1.1939x; 1.1939x over previous
"""Conv2d(128->256, 3x3, pad=1) + sync-BatchNorm(train) + ReLU on 8 TRN2 cores.

Strategy (data-parallel, hardcoded for x:[32,128,56,56] w:[256,128,3,3]):
  - Shard batch 32 -> 4 images/core across 8 cores.
  - Host pre-pads x to 58x58 (fp16) and pre-transposes weights to
    [Cin, o_tile, tap, o] (fp16, contiguous 128-col weight slices -> FWL).
  - Conv = implicit GEMM: Cin=128 is the partition/contraction dim; each 3x3 tap
    is one fp16 matmul ([128,128] weights x [128,448] shifted-image view)
    accumulated in fp32 PSUM. Output rows in 7 groups of 8 rows (8*56=448).
  - BN train-mode: conv bias cancels exactly under BN; per-channel stats via
    DVE bn_stats on PSUM (count/mean/M2 in one pass) + bn_aggr.
  - Two-stage pipeline over the Cout halves: o=0's stats AllReduce, normalize
    and store overlap with o=1's matmuls; only o=1's tail is exposed.
  - Final: out = Relu(y*scale + shift) on ACT into an fp32 staging tile,
    DMA'd to DRAM.
"""

import numpy as np

import concourse.bass as bass
import concourse.mybir as mybir
import concourse.tile as tile
from concourse import bacc

F32 = mybir.dt.float32
F16 = mybir.dt.float16

N_CORES = 8
IMGS = 4            # images per core
CIN = 128
COUT = 256
H = W = 56
HP = WP = 58        # padded
NG = 7              # row-groups per image (8 rows each)
RG = 8              # rows per group
GROUP = RG * W      # 448
BANK = 512          # fp32 elems per PSUM bank
EPS = 1e-5

AF = mybir.ActivationFunctionType
ALU = mybir.AluOpType


def build_nc() -> bass.Bass:
    # Bacc (not raw Bass): its compile pipeline legalizes semaphore waits
    # (TRN2 allows at most one wait per instruction; matmul waits move to
    # ldweights / event-semaphore instructions).
    nc = bacc.Bacc()
    xp_d = nc.declare_dram_parameter("xp", [IMGS, CIN, HP, WP], F16, isOutput=False)
    wt_d = nc.declare_dram_parameter("wt", [CIN, 2, 9, 128], F16, isOutput=False)
    gb_d = nc.declare_dram_parameter("gb", [128, 4], F32, isOutput=False)
    out_d = nc.declare_dram_parameter("out", [IMGS, COUT, H, W], F32, isOutput=True)

    import os as _os
    no_ar = bool(_os.environ.get("CONVACT_NO_AR"))

    with tile.TileContext(nc) as tc:
        with (
            tc.tile_pool(name="const", bufs=1) as cpool,
            tc.tile_pool(name="psum", bufs=2, space="PSUM") as ppool,
            tc.tile_pool(name="ostg", bufs=2) as opool,
            tc.tile_pool(name="dram", bufs=1, space="DRAM") as dpool,
        ):
            Wt = cpool.tile([128, 2, 9, 128], F16)
            GB = cpool.tile([128, 4], F32)
            X = cpool.tile([128, IMGS, HP, WP], F16)
            Y = cpool.tile([128, 2, IMGS, NG, GROUP], F16)
            S6 = cpool.tile([128, 2, IMGS * NG, 6], F32)
            MV = cpool.tile([128, 2, 2], F32)
            P = cpool.tile([128, 2, 2], F32)
            G = cpool.tile([128, 2, 2], F32)
            sqm = cpool.tile([128, 2], F32)
            e8 = cpool.tile([128, 2], F32)
            v64 = cpool.tile([128, 2], F32)
            std8 = cpool.tile([128, 2], F32)
            inv = cpool.tile([128, 2], F32)
            sc = cpool.tile([128, 2], F32)
            sh = cpool.tile([128, 2], F32)
            t2 = cpool.tile([128, 2], F32)
            eps64T = cpool.tile([128, 1], F32)
            dummy = cpool.tile([128, 1], F32)
            arin0 = dpool.tile([128, 2], F32)
            arin1 = dpool.tile([128, 2], F32)
            arout0 = dpool.tile([128, 2], F32)
            arout1 = dpool.tile([128, 2], F32)
            arin = [arin0, arin1]
            arout = [arout0, arout1]

            # ---- loads (one queue: weights + image0 first => first MM ~5us) ----
            nc.sync.dma_start(Wt[:, :, :, :], wt_d[:, :, :, :])
            nc.sync.dma_start(X[:, 0, :, :], xp_d[0, :, :, :])
            nc.sync.dma_start(GB[:, :], gb_d[:, :])
            for n in range(1, IMGS):
                nc.sync.dma_start(X[:, n, :, :], xp_d[n, :, :, :])

            # warm the ACT table set that holds Rsqrt (Copy/Relu are fillers
            # in every set) so no table load lands mid-kernel
            nc.vector.memset(eps64T[:, :], 64.0 * EPS)
            nc.scalar.activation(dummy[:, :], eps64T[:, :], AF.Sqrt)

            chunks = [(0, 4), (4, 3)]  # (first group, n groups) -> 4+3 PSUM banks

            def conv_chunk(o, n, g0, ngr):
                ps = ppool.tile([128, 4, BANK], F32, tag="ps")
                for gg in range(ngr):
                    g = g0 + gg
                    for t in range(9):
                        kh, kw = divmod(t, 3)
                        rhs = X[:, n, g * RG + kh : g * RG + kh + RG, kw : kw + W]
                        nc.tensor.matmul(
                            ps[:, gg, 0:GROUP],
                            Wt[:, o, t, :],
                            rhs,
                            start=(t == 0),
                            stop=(t == 8),
                        )
                # evacuate PSUM -> Y: ACT for o=0, DVE for o=1 (frees ACT to
                # run o=0's normalize during o=1's matmul phase)
                if o == 0:
                    nc.scalar.activation(
                        Y[:, o, n, g0 : g0 + ngr, :], ps[:, 0:ngr, 0:GROUP], AF.Copy
                    )
                else:
                    nc.vector.tensor_copy(
                        Y[:, o, n, g0 : g0 + ngr, :], ps[:, 0:ngr, 0:GROUP]
                    )
                # per-group BN partial stats straight from PSUM (DVE one-pass)
                for gg in range(ngr):
                    nc.vector.bn_stats(
                        S6[:, o, n * NG + g0 + gg, :], ps[:, gg, 0:GROUP]
                    )

            def launch_ar(o):
                # per-core (mean, E[y^2]) -> AllReduce(sum) across 8 cores
                nc.vector.bn_aggr(MV[:, o, :], S6[:, o, :, :])
                nc.vector.tensor_mul(t2[:, o : o + 1], MV[:, o, 0:1], MV[:, o, 0:1])
                nc.vector.tensor_add(P[:, o, 1:2], MV[:, o, 1:2], t2[:, o : o + 1])
                nc.vector.tensor_copy(P[:, o, 0:1], MV[:, o, 0:1])
                if no_ar:
                    nc.vector.tensor_scalar_mul(P[:, o, :], P[:, o, :], float(N_CORES))
                    nc.gpsimd.dma_start(arin[o][:, :], P[:, o, :])
                    nc.gpsimd.dma_start(arout[o][:, :], arin[o][:, :])
                else:
                    nc.gpsimd.dma_start(arin[o][:, :], P[:, o, :])
                    nc.gpsimd.collective_compute(
                        "AllReduce",
                        ALU.add,
                        replica_groups=[list(range(N_CORES))],
                        ins=[arin[o].opt()],
                        outs=[arout[o].opt()],
                    )
                nc.gpsimd.dma_start(G[:, o, :], arout[o][:, :])

            def finalize_a(o):
                # G[:,o] = (sum_c mean_c, sum_c E2_c);  var*64 = 8*sumE2 - summean^2
                nc.vector.tensor_mul(sqm[:, o : o + 1], G[:, o, 0:1], G[:, o, 0:1])
                nc.vector.tensor_scalar_mul(e8[:, o : o + 1], G[:, o, 1:2], 8.0)
                nc.vector.tensor_sub(v64[:, o : o + 1], e8[:, o : o + 1], sqm[:, o : o + 1])
                # sqrt(64*var + 64*eps) = 8 * sqrt(var+eps)
                nc.scalar.activation(
                    std8[:, o : o + 1], v64[:, o : o + 1], AF.Sqrt, bias=eps64T[:, 0:1]
                )

            def finalize_b(o):
                nc.vector.reciprocal(inv[:, o : o + 1], std8[:, o : o + 1])
                # GB holds 8*gamma -> sc = (8*gamma)/(8*std) = gamma*rsqrt(var+eps)
                nc.vector.tensor_mul(sc[:, o : o + 1], GB[:, o : o + 1], inv[:, o : o + 1])
                # sh = beta - mean*sc;  mean = summean/8
                nc.vector.tensor_mul(t2[:, o : o + 1], G[:, o, 0:1], sc[:, o : o + 1])
                nc.vector.tensor_scalar_mul(t2[:, o : o + 1], t2[:, o : o + 1], -0.125)
                nc.vector.tensor_add(sh[:, o : o + 1], GB[:, 2 + o : 3 + o], t2[:, o : o + 1])

            def relu_store(o, n):
                ob = opool.tile([128, H * W], F32, tag="ob")
                nc.scalar.activation(
                    ob[:, :],
                    Y[:, o, n, :, :].rearrange("p a b -> p (a b)"),
                    AF.Relu,
                    bias=sh[:, o : o + 1],
                    scale=sc[:, o : o + 1],
                )
                nc.sync.dma_start(
                    out_d[n, o * 128 : (o + 1) * 128, :, :].rearrange(
                        "p h w -> p (h w)"
                    ),
                    ob[:, :],
                )

            # ---- o=0 conv, then launch its AllReduce ----
            for n in range(IMGS):
                for g0, ngr in chunks:
                    conv_chunk(0, n, g0, ngr)
            launch_ar(0)

            # ---- o=1 conv with o=0's finalize/normalize interleaved ----
            for n in range(IMGS):
                for g0, ngr in chunks:
                    conv_chunk(1, n, g0, ngr)
                if n == 1:
                    finalize_a(0)   # DVE+ACT ops, wait on AR#1 result
                if n == 2:
                    finalize_b(0)
                    for nn in range(IMGS):
                        relu_store(0, nn)
            launch_ar(1)

            # ---- o=1 finalize + normalize + store (exposed tail) ----
            finalize_a(1)
            finalize_b(1)
            for n in range(IMGS):
                relu_store(1, n)
    return nc


_CACHE: dict = {}


def _get_nc() -> bass.Bass:
    if "nc" not in _CACHE:
        nc = build_nc()
        # Bacc.finalize runs the compile pipeline (wait legalization, register
        # allocation, nop fusion) - required before handing BIR to walrus.
        nc.finalize()
        _CACHE["nc"] = nc
    return _CACHE["nc"]


def _prep_inputs(x, weight, gamma, beta):
    x = np.asarray(x, dtype=np.float32)
    w = np.asarray(weight, dtype=np.float32)
    gamma = np.asarray(gamma, dtype=np.float32)
    beta = np.asarray(beta, dtype=np.float32)

    B = x.shape[0]
    per = B // N_CORES
    xp = np.zeros((B, CIN, HP, WP), np.float16)
    xp[:, :, 1 : 1 + H, 1 : 1 + W] = x
    # [Cout,Cin,3,3] -> [Cin, tap, Cout] -> [Cin, tap, o, 128] -> [Cin, o, tap, 128]
    wt = np.ascontiguousarray(
        w.transpose(1, 2, 3, 0).reshape(CIN, 9, 2, 128).transpose(0, 2, 1, 3),
        dtype=np.float16,
    )
    gb = np.ascontiguousarray(
        np.stack(
            [8.0 * gamma[:128], 8.0 * gamma[128:], beta[:128], beta[128:]], axis=1
        ),
        dtype=np.float32,
    )
    return [
        {"xp": xp[c * per : (c + 1) * per], "wt": wt, "gb": gb}
        for c in range(N_CORES)
    ]


def run(x, weight, bias=None, gamma=None, beta=None, trace=False, **kw):
    """Full-input entry; returns (out, BassKernelResults)."""
    from concourse.bass_utils import run_bass_kernel_spmd

    in_maps = _prep_inputs(x, weight, gamma, beta)
    res = run_bass_kernel_spmd(
        _get_nc(), in_maps, list(range(N_CORES)), trace=trace, **kw
    )
    out = np.concatenate([res.results[c]["out"] for c in range(N_CORES)], axis=0)
    return out, res


def kernel(x, weight, bias=None, gamma=None, beta=None):
    out, _ = run(x, weight, bias=bias, gamma=gamma, beta=beta, trace=False)
    return out


# revision 8
# speedup vs baseline: 1.3131x; 1.0999x over previous
"""Conv2d(128->256, 3x3, pad=1) + sync-BatchNorm(train) + ReLU on 8 TRN2 cores.

Strategy (data-parallel, hardcoded for x:[32,128,56,56] w:[256,128,3,3]):
  - Shard batch 32 -> 4 images/core across 8 cores.
  - Host pre-pads x to 58x58 (fp16) and pre-transposes weights to
    [Cin, o_tile, tap, o] (fp16, contiguous 128-col weight slices -> FWL).
  - Conv = implicit GEMM: Cin=128 is the partition/contraction dim; each 3x3 tap
    is one fp16 matmul ([128,128] weights x [128,448] shifted-image view)
    accumulated in fp32 PSUM. Output rows in 7 groups of 8 rows (8*56=448).
  - BN train-mode: conv bias cancels exactly under BN; per-channel stats via
    DVE bn_stats on PSUM (count/mean/M2 in one pass) + bn_aggr.
  - Two-stage pipeline over the Cout halves: o=0's stats AllReduce, normalize
    and store overlap with o=1's matmuls; only o=1's tail is exposed.
  - Final: out = Relu(y*scale + shift) on ACT into an fp32 staging tile,
    DMA'd to DRAM.
"""

import numpy as np

import concourse.bass as bass
import concourse.mybir as mybir
import concourse.tile as tile
from concourse import bacc

F32 = mybir.dt.float32
F16 = mybir.dt.float16

N_CORES = 8
IMGS = 4            # images per core
CIN = 128
COUT = 256
H = W = 56
HP = WP = 58        # padded
NG = 7              # row-groups per image (8 rows each)
RG = 8              # rows per group
GROUP = RG * W      # 448
BANK = 512          # fp32 elems per PSUM bank
EPS = 1e-5

AF = mybir.ActivationFunctionType
ALU = mybir.AluOpType


def build_nc() -> bass.Bass:
    # Bacc (not raw Bass): its compile pipeline legalizes semaphore waits
    # (TRN2 allows at most one wait per instruction; matmul waits move to
    # ldweights / event-semaphore instructions).
    nc = bacc.Bacc()
    xp_d = nc.declare_dram_parameter("xp", [IMGS, CIN, HP, WP], F16, isOutput=False)
    wt_d = nc.declare_dram_parameter("wt", [CIN, 2, 9, 128], F16, isOutput=False)
    gb_d = nc.declare_dram_parameter("gb", [128, 4], F32, isOutput=False)
    out_d = nc.declare_dram_parameter("out", [IMGS, COUT, H, W], F32, isOutput=True)

    import os as _os
    no_ar = bool(_os.environ.get("CONVACT_NO_AR"))

    with tile.TileContext(nc) as tc:
        with (
            tc.tile_pool(name="const", bufs=1) as cpool,
            tc.tile_pool(name="psum", bufs=2, space="PSUM") as ppool,
            tc.tile_pool(name="ostg", bufs=2) as opool,
            tc.tile_pool(name="dram", bufs=1, space="DRAM") as dpool,
        ):
            Wt = cpool.tile([128, 2, 9, 128], F16)
            GB = cpool.tile([128, 4], F32)
            X = cpool.tile([128, IMGS, HP, WP], F16)
            Y = cpool.tile([128, 2, IMGS, NG, GROUP], F16)
            S6 = cpool.tile([128, 2, IMGS * NG, 6], F32)
            MV = cpool.tile([128, 2, 2], F32)
            P = cpool.tile([128, 2, 2], F32)
            G = cpool.tile([128, 2, 2], F32)
            sqm = cpool.tile([128, 2], F32)
            e8 = cpool.tile([128, 2], F32)
            v64 = cpool.tile([128, 2], F32)
            std8 = cpool.tile([128, 2], F32)
            inv = cpool.tile([128, 2], F32)
            sc = cpool.tile([128, 2], F32)
            sh = cpool.tile([128, 2], F32)
            t2 = cpool.tile([128, 2], F32)
            eps64T = cpool.tile([128, 1], F32)
            dummy = cpool.tile([128, 1], F32)
            arin0 = dpool.tile([128, 2], F32)
            arin1 = dpool.tile([128, 2], F32)
            arout0 = dpool.tile([128, 2], F32)
            arout1 = dpool.tile([128, 2], F32)
            arin = [arin0, arin1]
            arout = [arout0, arout1]

            # ---- loads (one queue: weights + image0 first => first MM ~5us) ----
            nc.sync.dma_start(Wt[:, :, :, :], wt_d[:, :, :, :])
            nc.sync.dma_start(X[:, 0, :, :], xp_d[0, :, :, :])
            nc.sync.dma_start(GB[:, :], gb_d[:, :])
            for n in range(1, IMGS):
                nc.sync.dma_start(X[:, n, :, :], xp_d[n, :, :, :])

            # warm the ACT table set that holds Rsqrt (Copy/Relu are fillers
            # in every set) so no table load lands mid-kernel
            nc.vector.memset(eps64T[:, :], 64.0 * EPS)
            nc.scalar.activation(dummy[:, :], eps64T[:, :], AF.Sqrt)

            chunks = [(0, 4), (4, 3)]  # (first group, n groups) -> 4+3 PSUM banks

            def conv_chunk(o, n, g0, ngr):
                ps = ppool.tile([128, 4, BANK], F32, tag="ps")
                for gg in range(ngr):
                    g = g0 + gg
                    for t in range(9):
                        kh, kw = divmod(t, 3)
                        rhs = X[:, n, g * RG + kh : g * RG + kh + RG, kw : kw + W]
                        nc.tensor.matmul(
                            ps[:, gg, 0:GROUP],
                            Wt[:, o, t, :],
                            rhs,
                            start=(t == 0),
                            stop=(t == 8),
                        )
                # evacuate PSUM -> Y: ACT for o=0, DVE for o=1 (frees ACT to
                # run o=0's normalize during o=1's matmul phase)
                if o == 0:
                    nc.scalar.activation(
                        Y[:, o, n, g0 : g0 + ngr, :], ps[:, 0:ngr, 0:GROUP], AF.Copy
                    )
                else:
                    nc.vector.tensor_copy(
                        Y[:, o, n, g0 : g0 + ngr, :], ps[:, 0:ngr, 0:GROUP]
                    )
                # per-group BN partial stats straight from PSUM (DVE one-pass)
                for gg in range(ngr):
                    nc.vector.bn_stats(
                        S6[:, o, n * NG + g0 + gg, :], ps[:, gg, 0:GROUP]
                    )

            def launch_ar(o):
                # per-core (mean, E[y^2]) -> AllReduce(sum) across 8 cores
                nc.vector.bn_aggr(MV[:, o, :], S6[:, o, :, :])
                nc.vector.tensor_mul(t2[:, o : o + 1], MV[:, o, 0:1], MV[:, o, 0:1])
                nc.vector.tensor_add(P[:, o, 1:2], MV[:, o, 1:2], t2[:, o : o + 1])
                nc.vector.tensor_copy(P[:, o, 0:1], MV[:, o, 0:1])
                if no_ar:
                    nc.vector.tensor_scalar_mul(P[:, o, :], P[:, o, :], float(N_CORES))
                    nc.gpsimd.dma_start(arin[o][:, :], P[:, o, :])
                    nc.gpsimd.dma_start(arout[o][:, :], arin[o][:, :])
                else:
                    nc.gpsimd.dma_start(arin[o][:, :], P[:, o, :])
                    nc.gpsimd.collective_compute(
                        "AllReduce",
                        ALU.add,
                        replica_groups=[list(range(N_CORES))],
                        ins=[arin[o].opt()],
                        outs=[arout[o].opt()],
                    )
                nc.gpsimd.dma_start(G[:, o, :], arout[o][:, :])

            def finalize_a(o):
                # G[:,o] = (sum_c mean_c, sum_c E2_c);  var*64 = 8*sumE2 - summean^2
                # on GpSimd: it is already serialized behind the AR readback,
                # so the AR-wait never blocks the busy DVE/ACT queues
                nc.gpsimd.tensor_mul(sqm[:, o : o + 1], G[:, o, 0:1], G[:, o, 0:1])
                nc.gpsimd.tensor_scalar_mul(e8[:, o : o + 1], G[:, o, 1:2], 8.0)
                nc.gpsimd.tensor_sub(v64[:, o : o + 1], e8[:, o : o + 1], sqm[:, o : o + 1])
                # sqrt(64*var + 64*eps) = 8 * sqrt(var+eps)
                nc.scalar.activation(
                    std8[:, o : o + 1], v64[:, o : o + 1], AF.Sqrt, bias=eps64T[:, 0:1]
                )

            def finalize_b(o):
                nc.vector.reciprocal(inv[:, o : o + 1], std8[:, o : o + 1])
                # GB holds 8*gamma -> sc = (8*gamma)/(8*std) = gamma*rsqrt(var+eps)
                nc.vector.tensor_mul(sc[:, o : o + 1], GB[:, o : o + 1], inv[:, o : o + 1])
                # sh = beta - mean*sc;  mean = summean/8
                nc.vector.tensor_mul(t2[:, o : o + 1], G[:, o, 0:1], sc[:, o : o + 1])
                nc.vector.tensor_scalar_mul(t2[:, o : o + 1], t2[:, o : o + 1], -0.125)
                nc.vector.tensor_add(sh[:, o : o + 1], GB[:, 2 + o : 3 + o], t2[:, o : o + 1])

            def relu_store(o, n):
                ob = opool.tile([128, H * W], F32, tag="ob")
                nc.scalar.activation(
                    ob[:, :],
                    Y[:, o, n, :, :].rearrange("p a b -> p (a b)"),
                    AF.Relu,
                    bias=sh[:, o : o + 1],
                    scale=sc[:, o : o + 1],
                )
                nc.sync.dma_start(
                    out_d[n, o * 128 : (o + 1) * 128, :, :].rearrange(
                        "p h w -> p (h w)"
                    ),
                    ob[:, :],
                )

            # ---- o=0 conv, then launch its AllReduce ----
            for n in range(IMGS):
                for g0, ngr in chunks:
                    conv_chunk(0, n, g0, ngr)
            launch_ar(0)

            # ---- o=1 conv (pure: no AR-dependent op sits in front of the
            # evac/stats work in any busy engine queue) ----
            for n in range(IMGS):
                for g0, ngr in chunks:
                    conv_chunk(1, n, g0, ngr)

            # o=0 finalize + normalize + store: overlaps the last part of the
            # o=1 matmul phase and hides AR#2's peer-skew wait
            finalize_a(0)
            finalize_b(0)
            for n in range(IMGS):
                relu_store(0, n)
            launch_ar(1)

            # ---- o=1 finalize + normalize + store (exposed tail) ----
            finalize_a(1)
            finalize_b(1)
            for n in range(IMGS):
                relu_store(1, n)
    return nc


_CACHE: dict = {}


def _get_nc() -> bass.Bass:
    if "nc" not in _CACHE:
        nc = build_nc()
        # Bacc.finalize runs the compile pipeline (wait legalization, register
        # allocation, nop fusion) - required before handing BIR to walrus.
        nc.finalize()
        _CACHE["nc"] = nc
    return _CACHE["nc"]


def _prep_inputs(x, weight, gamma, beta):
    x = np.asarray(x, dtype=np.float32)
    w = np.asarray(weight, dtype=np.float32)
    gamma = np.asarray(gamma, dtype=np.float32)
    beta = np.asarray(beta, dtype=np.float32)

    B = x.shape[0]
    per = B // N_CORES
    xp = np.zeros((B, CIN, HP, WP), np.float16)
    xp[:, :, 1 : 1 + H, 1 : 1 + W] = x
    # [Cout,Cin,3,3] -> [Cin, tap, Cout] -> [Cin, tap, o, 128] -> [Cin, o, tap, 128]
    wt = np.ascontiguousarray(
        w.transpose(1, 2, 3, 0).reshape(CIN, 9, 2, 128).transpose(0, 2, 1, 3),
        dtype=np.float16,
    )
    gb = np.ascontiguousarray(
        np.stack(
            [8.0 * gamma[:128], 8.0 * gamma[128:], beta[:128], beta[128:]], axis=1
        ),
        dtype=np.float32,
    )
    return [
        {"xp": xp[c * per : (c + 1) * per], "wt": wt, "gb": gb}
        for c in range(N_CORES)
    ]


def run(x, weight, bias=None, gamma=None, beta=None, trace=False, **kw):
    """Full-input entry; returns (out, BassKernelResults)."""
    from concourse.bass_utils import run_bass_kernel_spmd

    in_maps = _prep_inputs(x, weight, gamma, beta)
    res = run_bass_kernel_spmd(
        _get_nc(), in_maps, list(range(N_CORES)), trace=trace, **kw
    )
    out = np.concatenate([res.results[c]["out"] for c in range(N_CORES)], axis=0)
    return out, res


def kernel(x, weight, bias=None, gamma=None, beta=None):
    out, _ = run(x, weight, bias=bias, gamma=gamma, beta=beta, trace=False)
    return out
